# revision 13
# baseline (speedup 1.0000x reference)
"""Distributed Trainium2 Bass kernel for the associative-embedding (AE) loss.

Problem: per image b (B=8), two tag maps (tm0 [J,256,256], tm1 [J,512,512]),
keypoints kps [NH, 3*J] (x, y, vis interleaved, NH=30 humans, J=17 joints).
Per level: gather tag values at (j, x, y), masked per-human mean, pull loss
(masked squared deviation / num_humans) + push loss (pairwise Gaussian of
means / num_humans^2).  Output: per-image loss [B] (sum over both levels).

Strategy: pure data-parallel over B across 8 NeuronCores (core b handles
image b).  The loss touches only NH*J = 510 elements of each tag map, so
instead of streaming the 178 MB of tag maps, each core computes flat gather
indices on-chip from the keypoint data and fetches the 1020 scalars with
three windowed `dma_gather` ext-ISA instructions (1024 indices each, 64-f32
rows; int16 row indices limit a window to 32768 rows = 2^21 elements, so
the 5.57M-element concatenated map needs 3 windows).  Out-of-window
elements fetch a clamped row and are discarded by the on-chip one-hot
select (row remainder never matches), making the three per-window selects
sum to exactly one hit per element.  The selected values live in a
[128, 8] chunk layout; one-hot matrices passed from the host let the
tensor engine reduce that layout directly into per-human sufficient
statistics (sum of masked vals, sum of masked vals^2), since
pull = sum(m v^2) - sv*avg.  The push loss uses a 32x32 DVE stream
transpose for the pairwise mean differences.  Per-core output is a single
scalar; the host stacks the 8 scalars into the final [8] vector.
"""

import numpy as np

B = 8
NH = 30
J = 17
H0 = W0 = 256
H1 = W1 = 512
N0 = J * H0 * W0
N1 = J * H1 * W1
NTOT = N0 + N1
NR = 2 * J * NH           # 1020 gathered elements
NC = 8                    # chunks of 128 (r = c*128 + p)
NI = 128 * NC             # 1024 padded indices
EL = 64                   # dma_gather row length (f32)
# gather windows: (element base, row count); rows capped at 2^15 (int16)
WINDOWS = [
    (0, 32768),
    (32768 * EL, 32768),
    (2 * 32768 * EL, (NTOT - 2 * 32768 * EL + EL - 1) // EL),
]
BIG = 1.0e9               # pad avg rows 30/31 -> exp(-BIG^2/2) = 0

_CACHE = {}
_GATHER_MODE = "gather"

# ---------------------------------------------------------------------------
# host-side constant/layout builders
# r = c*128 + p   (chunk layout: used by gather output, stats matmuls)
# r = s*16  + q   (wrapped layout: dma_gather index tiles, replicated x8)
# ---------------------------------------------------------------------------


def _host_constants():
    if "consts" in _CACHE:
        return _CACHE["consts"]
    r = np.arange(NI)
    valid = r < NR
    f = np.where(valid, r // NH, 0)
    nh = np.where(valid, r % NH, 0)
    lvl = f // J
    j = f % J
    wmul = np.where(valid, np.where(lvl == 0, W0, W1), 0)
    base = np.where(valid, np.where(lvl == 0, j * H0 * W0, N0 + j * H1 * W1), 0)

    def chunkify(a):  # [NI] -> [128, NC]  (col c, row p = r = c*128+p)
        return np.ascontiguousarray(a.reshape(NC, 128).T)

    def wrapify(a):  # [NI] -> [128, NI//16] replicated over 8 groups
        w = np.ascontiguousarray(a.reshape(NI // 16, 16).T)  # [16, NI//16]
        return np.tile(w, (8, 1))

    kcw = chunkify(wmul).astype(np.int32)
    kcb = chunkify(base).astype(np.int32)
    kww = wrapify(wmul).astype(np.int32)
    kwb = wrapify(base).astype(np.int32)

    L0 = chunkify((valid & (lvl == 0)).astype(np.float32))
    L1 = chunkify((valid & (lvl == 1)).astype(np.float32))
    E = np.zeros((128, NC * NH), dtype=np.float32)
    for c in range(NC):
        rr = np.arange(c * 128, (c + 1) * 128)
        ok = rr < NR
        E[ok, c * NH + (rr[ok] % NH)] = 1.0
    io = np.tile(np.arange(EL, dtype=np.float32), NC)[None, :].repeat(128, 0)
    cf = np.concatenate([L0, L1, E, io], axis=1).astype(np.float32)

    col_x = (lvl * 3 * J + 3 * j).astype(np.int64)
    _CACHE["consts"] = dict(
        kcw=kcw, kcb=kcb, kww=kww, kwb=kwb, cf=cf,
        nh=nh, col_x=col_x, valid=valid,
        chunkify=chunkify, wrapify=wrapify,
    )
    return _CACHE["consts"]


def make_in_maps(tag_maps0, tag_maps1, kps0, kps1):
    tag_maps0 = np.asarray(tag_maps0, dtype=np.float32)
    tag_maps1 = np.asarray(tag_maps1, dtype=np.float32)
    kps0 = np.asarray(kps0, dtype=np.int32)
    kps1 = np.asarray(kps1, dtype=np.int32)
    C = _host_constants()
    nh, col_x, valid = C["nh"], C["col_x"], C["valid"]
    chunkify, wrapify = C["chunkify"], C["wrapify"]
    in_maps = []
    for b in range(B):
        tm = np.concatenate(
            [tag_maps0[b].ravel(), tag_maps1[b].ravel()]
        ).reshape(NTOT, 1)
        kp = np.concatenate([kps0[b], kps1[b]], axis=1)  # [30, 102]
        # pure relayouts of kp into the two device layouts
        xs = np.zeros(NI, np.int32)
        ys = np.zeros(NI, np.int32)
        vs = np.zeros(NI, np.int32)
        xs[valid] = kp[nh[valid], col_x[valid]]
        ys[valid] = kp[nh[valid], col_x[valid] + 1]
        vs[valid] = kp[nh[valid], col_x[valid] + 2]
        kpg = np.stack(
            [chunkify(xs), chunkify(ys), chunkify(vs)], axis=2
        ).reshape(128, 3 * NC)
        ki = np.concatenate([kpg, C["kcw"], C["kcb"]], axis=1)  # [128, 40]
        kw = np.concatenate(
            [wrapify(xs), wrapify(ys), C["kww"], C["kwb"]], axis=1
        )  # [128, 256]
        in_maps.append({"tm": tm, "kp": kp, "ki": ki, "kw": kw, "cf": C["cf"]})
    return in_maps


# ---------------------------------------------------------------------------
# device kernel
# ---------------------------------------------------------------------------


def _build_nc():
    from concourse import bacc, mybir
    import concourse.tile as tile
    from concourse.library_config import mlp

    f32 = mybir.dt.float32
    i32 = mybir.dt.int32
    i16 = mybir.dt.int16
    Alu = mybir.AluOpType
    X = mybir.AxisListType.X
    NW = NI // 16  # wrapped-layout column count (64)

    nc = bacc.Bacc()
    TM = nc.declare_dram_parameter("tm", [NTOT, 1], f32, isOutput=False)
    KP = nc.declare_dram_parameter("kp", [NH, 6 * J], i32, isOutput=False)
    KI = nc.declare_dram_parameter("ki", [128, 5 * NC], i32, isOutput=False)
    KW = nc.declare_dram_parameter("kw", [128, 4 * NW], i32, isOutput=False)
    CF = nc.declare_dram_parameter(
        "cf", [128, 2 * NC + NC * NH + NC * EL], f32, isOutput=False
    )
    OUT = nc.declare_dram_parameter("out", [1, 1], f32, isOutput=True)

    with tile.TileContext(nc) as tc:
        with (
            tc.tile_pool(name="sb", bufs=1) as sb,
            tc.tile_pool(name="pp", bufs=1, space="PSUM") as pp,
        ):
            kt = sb.tile([NH, 6 * J], i32)
            ki = sb.tile([128, 5 * NC], i32)
            kw = sb.tile([128, 4 * NW], i32)
            cf = sb.tile([128, 2 * NC + NC * NH + NC * EL], f32)
            idxc = sb.tile([128, NC], i32)
            idxw = sb.tile([128, NW], i32)
            V = [sb.tile([128, NC * EL], f32, name=f"V{w}", tag=f"V{w}") for w in range(3)]
            ix16 = [sb.tile([128, NW], i16, name=f"ix{w}", tag=f"ix{w}") for w in range(3)]
            remf = [sb.tile([128, NC], f32, name=f"rm{w}", tag=f"rm{w}") for w in range(3)]
            rt = sb.tile([128, NW], i32)
            six = sb.tile([128, NW], i32)
            rc = sb.tile([128, NC], i32)
            uc = sb.tile([128, NC], i32)
            oneh = sb.tile([128, NC * EL], f32)
            sel = [sb.tile([128, NC], f32, name=f"sl{w}", tag=f"sl{w}") for w in range(3)]
            S = sb.tile([128, NC], f32)
            maskg = sb.tile([128, NC], f32)
            Sg = sb.tile([128, NC], f32)
            Sg2 = sb.tile([128, NC], f32)
            T = sb.tile([128, 4 * NC], f32)
            maskf = sb.tile([NH, 2 * J], f32)
            cnt = sb.tile([NH, 2], f32)
            den = sb.tile([NH, 2], f32)
            rden = sb.tile([NH, 2], f32)
            st = sb.tile([NH, 4], f32)
            avg0 = sb.tile([NH, 2], f32)
            u = sb.tile([NH, 2], f32)
            avg32 = sb.tile([32, 2], f32)
            avgsrc = sb.tile([32, 64], f32)
            avgT = sb.tile([32, 64], f32)
            d2 = sb.tile([NH, 64], f32)
            pm = sb.tile([NH, 64], f32)
            pack = sb.tile([NH, 6], f32)
            ones = sb.tile([NH, 1], f32)
            warm = sb.tile([1, 1], f32)
            sums = sb.tile([1, 6], f32)
            rec = sb.tile([1, 6], f32)
            m1 = sb.tile([1, 4], f32)
            res = sb.tile([1, 1], f32)
            ps_st = pp.tile([NH, 4], f32)
            ps_f = pp.tile([1, 6], f32)

            # Prefetch the Q7 ext-ISA library + ACT Exp table during startup.
            nc.gpsimd.load_library(mlp)
            nc.vector.memset(warm[:], 0.0)
            nc.scalar.activation(
                warm[:], warm[:], mybir.ActivationFunctionType.Exp
            )

            # Inputs in (ki/kw gate the gathers: issue first, on both HWDGE
            # engines).
            nc.sync.dma_start(ki[:], KI[:])
            nc.sync.dma_start(kw[:], KW[:])
            nc.sync.dma_start(kt[:], KP[:])
            nc.sync.dma_start(cf[:], CF[:])

            # Flat element index in both layouts: idx = x*W + y + base.
            xg = ki[:, 0 : 3 * NC : 3]
            yg = ki[:, 1 : 3 * NC : 3]
            vg = ki[:, 2 : 3 * NC : 3]
            nc.vector.tensor_tensor(
                out=idxc[:], in0=xg, in1=ki[:, 3 * NC : 4 * NC], op=Alu.mult
            )
            nc.vector.tensor_tensor(out=idxc[:], in0=idxc[:], in1=yg, op=Alu.add)
            nc.vector.tensor_tensor(
                out=idxc[:], in0=idxc[:], in1=ki[:, 4 * NC : 5 * NC], op=Alu.add
            )
            nc.vector.tensor_tensor(
                out=idxw[:],
                in0=kw[:, 0:NW],
                in1=kw[:, 2 * NW : 3 * NW],
                op=Alu.mult,
            )
            nc.vector.tensor_tensor(
                out=idxw[:], in0=idxw[:], in1=kw[:, NW : 2 * NW], op=Alu.add
            )
            nc.vector.tensor_tensor(
                out=idxw[:], in0=idxw[:], in1=kw[:, 3 * NW : 4 * NW], op=Alu.add
            )

            # Joint-count path (independent of the gathers; runs early).
            vis = kt[:, 2 : 6 * J : 3]
            nc.vector.tensor_scalar(
                out=maskf[:], in0=vis, scalar1=0, scalar2=None, op0=Alu.is_gt
            )
            nc.vector.reduce_sum(
                out=cnt[:], in_=maskf[:].rearrange("p (l j) -> p l j", l=2), axis=X
            )
            nc.vector.tensor_scalar(
                out=den[:], in0=cnt[:], scalar1=1.0, scalar2=None, op0=Alu.max
            )
            nc.vector.reciprocal(rden[:], den[:])
            nc.vector.tensor_scalar(
                out=pack[:, 4:6], in0=cnt[:], scalar1=0.0, scalar2=None, op0=Alu.is_gt
            )
            nc.vector.tensor_scalar(
                out=maskg[:], in0=vg, scalar1=0, scalar2=None, op0=Alu.is_gt
            )

            # Per-window row indices (wrapped, int16) + remainders (chunk).
            nc.vector.memset(six[:], 6)
            for w, (wb, rows) in enumerate(WINDOWS):
                nc.vector.tensor_scalar(
                    out=rt[:], in0=idxw[:], scalar1=wb, scalar2=None,
                    op0=Alu.subtract,
                )
                nc.vector.tensor_tensor(
                    out=rt[:], in0=rt[:], in1=six[:], op=Alu.arith_shift_right
                )
                nc.vector.tensor_scalar(
                    out=rt[:], in0=rt[:], scalar1=0, scalar2=rows - 1,
                    op0=Alu.max, op1=Alu.min,
                )
                nc.vector.tensor_copy(out=ix16[w][:], in_=rt[:])
                nc.vector.tensor_scalar(
                    out=rc[:], in0=idxc[:], scalar1=wb, scalar2=None,
                    op0=Alu.subtract,
                )
                nc.vector.tensor_tensor(
                    out=rc[:], in0=rc[:], in1=six[:, 0:NC], op=Alu.arith_shift_right
                )
                nc.vector.tensor_scalar(
                    out=rc[:], in0=rc[:], scalar1=0, scalar2=rows - 1,
                    op0=Alu.max, op1=Alu.min,
                )
                nc.vector.tensor_scalar(
                    out=uc[:], in0=rc[:], scalar1=EL, scalar2=wb,
                    op0=Alu.mult, op1=Alu.add,
                )
                nc.vector.tensor_tensor(
                    out=uc[:], in0=idxc[:], in1=uc[:], op=Alu.subtract
                )
                nc.vector.tensor_copy(out=remf[w][:], in_=uc[:])

            # Three windowed gathers + one-hot selects.
            iof = cf[:, 2 * NC + NC * NH :]
            for w, (wb, rows) in enumerate(WINDOWS):
                src = TM[wb : wb + rows * EL, :].rearrange(
                    "(r e) o -> r (e o)", e=EL
                )
                if _GATHER_MODE == "memset":
                    nc.vector.memset(V[w][:], 0.0)
                else:
                    nc.gpsimd.dma_gather(
                        V[w][:].rearrange("p (c e) -> p c e", c=NC),
                        src,
                        ix16[w][:],
                        NI,
                        NI,
                        EL,
                    )
                nc.vector.tensor_tensor(
                    out=oneh[:].rearrange("p (c e) -> p c e", c=NC),
                    in0=iof.rearrange("p (c e) -> p c e", c=NC),
                    in1=remf[w][:].to_broadcast([128, NC, EL]),
                    op=Alu.is_equal,
                )
                nc.vector.tensor_tensor(
                    out=oneh[:], in0=oneh[:], in1=V[w][:], op=Alu.mult
                )
                nc.vector.reduce_sum(
                    out=sel[w][:],
                    in_=oneh[:].rearrange("p (c e) -> p c e", c=NC),
                    axis=X,
                )
            nc.vector.tensor_tensor(
                out=S[:], in0=sel[0][:], in1=sel[1][:], op=Alu.add
            )
            nc.vector.tensor_tensor(out=S[:], in0=S[:], in1=sel[2][:], op=Alu.add)

            # Masked first/second moments -> per-human stats via one-hot
            # matmuls: st = [sv0, sv1, s2_0, s2_1].
            nc.vector.tensor_tensor(out=Sg[:], in0=S[:], in1=maskg[:], op=Alu.mult)
            nc.vector.tensor_tensor(out=Sg2[:], in0=Sg[:], in1=S[:], op=Alu.mult)
            L0 = cf[:, 0:NC]
            L1 = cf[:, NC : 2 * NC]
            nc.vector.tensor_tensor(out=T[:, 0:NC], in0=Sg[:], in1=L0, op=Alu.mult)
            nc.vector.tensor_tensor(
                out=T[:, NC : 2 * NC], in0=Sg[:], in1=L1, op=Alu.mult
            )
            nc.vector.tensor_tensor(
                out=T[:, 2 * NC : 3 * NC], in0=Sg2[:], in1=L0, op=Alu.mult
            )
            nc.vector.tensor_tensor(
                out=T[:, 3 * NC : 4 * NC], in0=Sg2[:], in1=L1, op=Alu.mult
            )
            for c in range(NC):
                nc.tensor.matmul(
                    ps_st[:],
                    lhsT=cf[:, 2 * NC + c * NH : 2 * NC + (c + 1) * NH],
                    rhs=T[:, c : 4 * NC : NC],
                    start=(c == 0),
                    stop=(c == NC - 1),
                )
            nc.vector.tensor_copy(out=st[:], in_=ps_st[:])

            sv = st[:, 0:2]
            s2 = st[:, 2:4]
            nc.vector.tensor_tensor(out=avg0[:], in0=sv, in1=rden[:], op=Alu.mult)
            nc.vector.memset(avg32[:], BIG)
            nc.vector.tensor_tensor(
                out=avg32[0:NH, :], in0=avg0[:], in1=pack[:, 4:6], op=Alu.mult
            )
            # pull = s2 - sv*avg0 (zero when cnt == 0 since sv = s2 = 0)
            nc.vector.tensor_tensor(out=u[:], in0=sv, in1=avg0[:], op=Alu.mult)
            nc.vector.tensor_tensor(
                out=pack[:, 0:2], in0=s2, in1=u[:], op=Alu.subtract
            )

            # Push: pairwise means via 32x32 block stream transpose.
            nc.vector.tensor_copy(
                out=avgsrc[:].rearrange("p (l j) -> p l j", l=2),
                in_=avg32[:].to_broadcast([32, 2, 32]),
            )
            nc.vector.transpose(avgT[:], avgsrc[:])
            nc.vector.tensor_tensor(
                out=d2[:].rearrange("p (l j) -> p l j", l=2),
                in0=avgT[0:NH, :].rearrange("p (l j) -> p l j", l=2),
                in1=avg32[0:NH, :].to_broadcast([NH, 2, 32]),
                op=Alu.subtract,
            )
            nc.vector.tensor_tensor(out=d2[:], in0=d2[:], in1=d2[:], op=Alu.mult)
            nc.scalar.activation(
                pm[:], d2[:], mybir.ActivationFunctionType.Exp, scale=-0.5
            )
            nc.vector.reduce_sum(
                out=pack[:, 2:4],
                in_=pm[:].rearrange("p (l j) -> p l j", l=2),
                axis=X,
            )

            # Column sums over the 30 humans via PE, then the final scalar.
            nc.vector.memset(ones[:], 1.0)
            nc.tensor.matmul(
                ps_f[:], lhsT=ones[:], rhs=pack[:], start=True, stop=True
            )
            nc.vector.tensor_copy(out=sums[:], in_=ps_f[:])
            nc.vector.reciprocal(rec[:], sums[:])
            nc.vector.tensor_tensor(
                out=m1[:, 0:2], in0=sums[:, 0:2], in1=rec[:, 4:6], op=Alu.mult
            )
            nc.vector.tensor_tensor(
                out=m1[:, 2:4], in0=sums[:, 2:4], in1=rec[:, 4:6], op=Alu.mult
            )
            nc.vector.tensor_tensor(
                out=m1[:, 2:4], in0=m1[:, 2:4], in1=rec[:, 4:6], op=Alu.mult
            )
            nc.vector.reduce_sum(out=res[:], in_=m1[:], axis=X)

            nc.sync.dma_start(OUT[:], res[:])

    nc.finalize()
    return nc


def _get_nc():
    if "nc" not in _CACHE:
        _CACHE["nc"] = _build_nc()
    return _CACHE["nc"]


def kernel(tag_maps0, tag_maps1, kps0, kps1):
    from concourse.bass_utils import run_bass_kernel_spmd

    nc = _get_nc()
    in_maps = make_in_maps(tag_maps0, tag_maps1, kps0, kps1)
    out = run_bass_kernel_spmd(nc, in_maps, core_ids=list(range(B)))
    return np.array(
        [np.asarray(out.results[b]["out"]).reshape(()) for b in range(B)],
        dtype=np.float32,
    )


# revision 14
# speedup vs baseline: 3.3003x; 3.3003x over previous
"""Distributed Trainium2 Bass kernel for the associative-embedding (AE) loss.

Problem: per image b (B=8), two tag maps (tm0 [J,256,256], tm1 [J,512,512]),
keypoints kps [NH, 3*J] (x, y, vis interleaved, NH=30 humans, J=17 joints).
Per level: gather tag values at (j, x, y), masked per-human mean, pull loss
(masked squared deviation / num_humans) + push loss (pairwise Gaussian of
means / num_humans^2).  Output: per-image loss [B] (sum over both levels).

Strategy: pure data-parallel over B across 8 NeuronCores (core b handles
image b).  The loss touches only NH*J = 510 elements of each tag map, so
instead of streaming the 178 MB of tag maps, each core computes flat gather
indices on-chip from the keypoint data and pulls exactly 1020 scalars out
of DRAM via 8 indirect (SWDGE) DMAs of 128 single-element descriptors each
(HW indirect DMA = one descriptor per out partition row).  The gathered
values live in a [128, 8] chunk layout; one-hot matrices passed from the
host let the tensor engine reduce that layout directly into per-human
sufficient statistics (sum of masked vals, sum of masked vals^2), since
pull = sum(m v^2) - sv*avg.  Per-chunk stat products and matmuls are
pipelined under the remaining gathers.  The push loss uses a 32x32 DVE
stream transpose for the pairwise mean differences.  Per-core output is a
single scalar; the host stacks the 8 scalars into the final [8] vector.
"""

import numpy as np

B = 8
NH = 30
J = 17
H0 = W0 = 256
H1 = W1 = 512
N0 = J * H0 * W0
N1 = J * H1 * W1
NTOT = N0 + N1
NR = 2 * J * NH           # 1020 gathered elements
NC = 8                    # chunks of 128 (r = c*128 + p)
NI = 128 * NC
BIG = 1.0e9               # pad avg rows 30/31 -> exp(-BIG^2/2) = 0

_CACHE = {}

# ---------------------------------------------------------------------------
# host-side constants: chunk layout r = c*128 + p, f = r // NH, nh = r % NH
# ---------------------------------------------------------------------------


def _host_constants():
    if "consts" in _CACHE:
        return _CACHE["consts"]
    r = np.arange(NI)
    valid = r < NR
    f = np.where(valid, r // NH, 0)
    nh = np.where(valid, r % NH, 0)
    lvl = f // J
    j = f % J
    wmul = np.where(valid, np.where(lvl == 0, W0, W1), 0)
    base = np.where(valid, np.where(lvl == 0, j * H0 * W0, N0 + j * H1 * W1), 0)

    def chunkify(a):  # [NI] -> [128, NC]
        return np.ascontiguousarray(a.reshape(NC, 128).T)

    kcw = chunkify(wmul).astype(np.int32)
    kcb = chunkify(base).astype(np.int32)
    L0 = chunkify((valid & (lvl == 0)).astype(np.float32))
    L1 = chunkify((valid & (lvl == 1)).astype(np.float32))
    E = np.zeros((128, NC * NH), dtype=np.float32)
    for c in range(NC):
        rr = np.arange(c * 128, (c + 1) * 128)
        ok = rr < NR
        E[ok, c * NH + (rr[ok] % NH)] = 1.0
    cf = np.concatenate([L0, L1, E], axis=1).astype(np.float32)
    col_x = (lvl * 3 * J + 3 * j).astype(np.int64)
    _CACHE["consts"] = dict(
        kcw=kcw, kcb=kcb, cf=cf, nh=nh, col_x=col_x, valid=valid,
        chunkify=chunkify,
    )
    return _CACHE["consts"]


def make_in_maps(tag_maps0, tag_maps1, kps0, kps1):
    tag_maps0 = np.asarray(tag_maps0, dtype=np.float32)
    tag_maps1 = np.asarray(tag_maps1, dtype=np.float32)
    kps0 = np.asarray(kps0, dtype=np.int32)
    kps1 = np.asarray(kps1, dtype=np.int32)
    C = _host_constants()
    nh, col_x, valid = C["nh"], C["col_x"], C["valid"]
    chunkify = C["chunkify"]
    in_maps = []
    for b in range(B):
        tm = np.concatenate(
            [tag_maps0[b].ravel(), tag_maps1[b].ravel()]
        ).reshape(NTOT, 1)
        kp = np.concatenate([kps0[b], kps1[b]], axis=1)  # [30, 102]
        xs = np.zeros(NI, np.int32)
        ys = np.zeros(NI, np.int32)
        vs = np.zeros(NI, np.int32)
        xs[valid] = kp[nh[valid], col_x[valid]]
        ys[valid] = kp[nh[valid], col_x[valid] + 1]
        vs[valid] = kp[nh[valid], col_x[valid] + 2]
        kpg = np.stack(
            [chunkify(xs), chunkify(ys), chunkify(vs)], axis=2
        ).reshape(128, 3 * NC)
        ki = np.concatenate([kpg, C["kcw"], C["kcb"]], axis=1)  # [128, 40]
        in_maps.append({"tm": tm, "kp": kp, "ki": ki, "cf": C["cf"]})
    return in_maps


# ---------------------------------------------------------------------------
# device kernel
# ---------------------------------------------------------------------------


def _build_nc():
    from concourse import bacc, mybir
    import concourse.tile as tile
    from concourse.bass import IndirectOffsetOnAxis

    f32 = mybir.dt.float32
    i32 = mybir.dt.int32
    Alu = mybir.AluOpType
    X = mybir.AxisListType.X

    nc = bacc.Bacc()
    TM = nc.declare_dram_parameter("tm", [NTOT, 1], f32, isOutput=False)
    KP = nc.declare_dram_parameter("kp", [NH, 6 * J], i32, isOutput=False)
    KI = nc.declare_dram_parameter("ki", [128, 5 * NC], i32, isOutput=False)
    CF = nc.declare_dram_parameter(
        "cf", [128, 2 * NC + NC * NH], f32, isOutput=False
    )
    OUT = nc.declare_dram_parameter("out", [1, 1], f32, isOutput=True)

    with tile.TileContext(nc) as tc:
        with (
            tc.tile_pool(name="sb", bufs=1) as sb,
            tc.tile_pool(name="pp", bufs=1, space="PSUM") as pp,
        ):
            kt = sb.tile([NH, 6 * J], i32)
            ki = sb.tile([128, 5 * NC], i32)
            cf = sb.tile([128, 2 * NC + NC * NH], f32)
            idxc = sb.tile([128, NC], i32)
            S = sb.tile([128, NC], f32)
            maskg = sb.tile([128, NC], f32)
            T = sb.tile([128, 4 * NC], f32)
            sg = sb.tile([128, NC], f32)
            maskf = sb.tile([NH, 2 * J], f32)
            cnt = sb.tile([NH, 2], f32)
            den = sb.tile([NH, 2], f32)
            rden = sb.tile([NH, 2], f32)
            st = sb.tile([NH, 4], f32)
            avg0 = sb.tile([NH, 2], f32)
            u = sb.tile([NH, 2], f32)
            avg32 = sb.tile([32, 2], f32)
            avgsrc = sb.tile([32, 64], f32)
            avgT = sb.tile([32, 64], f32)
            d2 = sb.tile([NH, 64], f32)
            pm = sb.tile([NH, 64], f32)
            pack = sb.tile([NH, 6], f32)
            ones = sb.tile([NH, 1], f32)
            warm = sb.tile([1, 1], f32)
            sums = sb.tile([1, 6], f32)
            rec = sb.tile([1, 6], f32)
            m1 = sb.tile([1, 4], f32)
            res = sb.tile([1, 1], f32)
            ps_st = pp.tile([NH, 4], f32)
            ps_f = pp.tile([1, 6], f32)

            # Warm the ACT Exp table during startup.
            nc.vector.memset(warm[:], 0.0)
            nc.scalar.activation(
                warm[:], warm[:], mybir.ActivationFunctionType.Exp
            )

            # Inputs in; ki first (it gates the gathers).
            nc.sync.dma_start(ki[:], KI[:])
            nc.sync.dma_start(kt[:], KP[:])
            nc.sync.dma_start(cf[:], CF[:])

            # Gather indices in chunk layout: idx = x*W + y + base.
            xg = ki[:, 0 : 3 * NC : 3]
            yg = ki[:, 1 : 3 * NC : 3]
            vg = ki[:, 2 : 3 * NC : 3]
            nc.vector.tensor_tensor(
                out=idxc[:], in0=xg, in1=ki[:, 3 * NC : 4 * NC], op=Alu.mult
            )
            nc.vector.tensor_tensor(out=idxc[:], in0=idxc[:], in1=yg, op=Alu.add)
            nc.vector.tensor_tensor(
                out=idxc[:], in0=idxc[:], in1=ki[:, 4 * NC : 5 * NC], op=Alu.add
            )
            nc.vector.tensor_scalar(
                out=maskg[:], in0=vg, scalar1=0, scalar2=None, op0=Alu.is_gt
            )

            # The only touch of the big tag maps: 1020 scalars in 8 indirect
            # DMAs; per-chunk stat products + matmuls pipeline right behind
            # each chunk's gather.
            L0 = cf[:, 0:NC]
            L1 = cf[:, NC : 2 * NC]
            for c in range(NC):
                nc.gpsimd.indirect_dma_start(
                    out=S[:, c : c + 1],
                    out_offset=None,
                    in_=TM[:],
                    in_offset=IndirectOffsetOnAxis(ap=idxc[:, c : c + 1], axis=0),
                )
                cs = slice(c, c + 1)
                nc.vector.tensor_tensor(
                    out=sg[:, cs], in0=S[:, cs], in1=maskg[:, cs], op=Alu.mult
                )
                nc.vector.tensor_tensor(
                    out=T[:, c : c + 1], in0=sg[:, cs], in1=L0[:, cs], op=Alu.mult
                )
                nc.vector.tensor_tensor(
                    out=T[:, NC + c : NC + c + 1],
                    in0=sg[:, cs],
                    in1=L1[:, cs],
                    op=Alu.mult,
                )
                nc.vector.tensor_tensor(
                    out=sg[:, cs], in0=sg[:, cs], in1=S[:, cs], op=Alu.mult
                )
                nc.vector.tensor_tensor(
                    out=T[:, 2 * NC + c : 2 * NC + c + 1],
                    in0=sg[:, cs],
                    in1=L0[:, cs],
                    op=Alu.mult,
                )
                nc.vector.tensor_tensor(
                    out=T[:, 3 * NC + c : 3 * NC + c + 1],
                    in0=sg[:, cs],
                    in1=L1[:, cs],
                    op=Alu.mult,
                )
                nc.tensor.matmul(
                    ps_st[:],
                    lhsT=cf[:, 2 * NC + c * NH : 2 * NC + (c + 1) * NH],
                    rhs=T[:, c : 4 * NC : NC],
                    start=(c == 0),
                    stop=(c == NC - 1),
                )

            # Joint-count path (independent of gathers; fills DVE idle time).
            vis = kt[:, 2 : 6 * J : 3]
            nc.vector.tensor_scalar(
                out=maskf[:], in0=vis, scalar1=0, scalar2=None, op0=Alu.is_gt
            )
            nc.vector.reduce_sum(
                out=cnt[:], in_=maskf[:].rearrange("p (l j) -> p l j", l=2), axis=X
            )
            nc.vector.tensor_scalar(
                out=den[:], in0=cnt[:], scalar1=1.0, scalar2=None, op0=Alu.max
            )
            nc.vector.reciprocal(rden[:], den[:])
            nc.vector.tensor_scalar(
                out=pack[:, 4:6], in0=cnt[:], scalar1=0.0, scalar2=None, op0=Alu.is_gt
            )
            nc.vector.memset(avg32[:], BIG)
            nc.vector.memset(ones[:], 1.0)

            # Per-human stats -> averages, pull.
            nc.vector.tensor_copy(out=st[:], in_=ps_st[:])
            sv = st[:, 0:2]
            s2 = st[:, 2:4]
            nc.vector.tensor_tensor(out=avg0[:], in0=sv, in1=rden[:], op=Alu.mult)
            nc.vector.tensor_tensor(
                out=avg32[0:NH, :], in0=avg0[:], in1=pack[:, 4:6], op=Alu.mult
            )
            # pull = s2 - sv*avg0 (zero when cnt == 0 since sv = s2 = 0)
            nc.vector.tensor_tensor(out=u[:], in0=sv, in1=avg0[:], op=Alu.mult)
            nc.vector.tensor_tensor(
                out=pack[:, 0:2], in0=s2, in1=u[:], op=Alu.subtract
            )

            # Push: pairwise means via 32x32 block stream transpose.
            nc.vector.tensor_copy(
                out=avgsrc[:].rearrange("p (l j) -> p l j", l=2),
                in_=avg32[:].to_broadcast([32, 2, 32]),
            )
            nc.vector.transpose(avgT[:], avgsrc[:])
            nc.vector.tensor_tensor(
                out=d2[:].rearrange("p (l j) -> p l j", l=2),
                in0=avgT[0:NH, :].rearrange("p (l j) -> p l j", l=2),
                in1=avg32[0:NH, :].to_broadcast([NH, 2, 32]),
                op=Alu.subtract,
            )
            nc.vector.tensor_tensor(out=d2[:], in0=d2[:], in1=d2[:], op=Alu.mult)
            nc.scalar.activation(
                pm[:], d2[:], mybir.ActivationFunctionType.Exp, scale=-0.5
            )
            nc.vector.reduce_sum(
                out=pack[:, 2:4],
                in_=pm[:].rearrange("p (l j) -> p l j", l=2),
                axis=X,
            )

            # Column sums over the 30 humans via PE, then the final scalar.
            nc.tensor.matmul(
                ps_f[:], lhsT=ones[:], rhs=pack[:], start=True, stop=True
            )
            nc.vector.tensor_copy(out=sums[:], in_=ps_f[:])
            nc.vector.reciprocal(rec[:], sums[:])
            nc.vector.tensor_tensor(
                out=m1[:, 0:2], in0=sums[:, 0:2], in1=rec[:, 4:6], op=Alu.mult
            )
            nc.vector.tensor_tensor(
                out=m1[:, 2:4], in0=sums[:, 2:4], in1=rec[:, 4:6], op=Alu.mult
            )
            nc.vector.tensor_tensor(
                out=m1[:, 2:4], in0=m1[:, 2:4], in1=rec[:, 4:6], op=Alu.mult
            )
            nc.vector.reduce_sum(out=res[:], in_=m1[:], axis=X)

            nc.sync.dma_start(OUT[:], res[:])

    nc.finalize()
    return nc


def _get_nc():
    if "nc" not in _CACHE:
        _CACHE["nc"] = _build_nc()
    return _CACHE["nc"]


def kernel(tag_maps0, tag_maps1, kps0, kps1):
    from concourse.bass_utils import run_bass_kernel_spmd

    nc = _get_nc()
    in_maps = make_in_maps(tag_maps0, tag_maps1, kps0, kps1)
    out = run_bass_kernel_spmd(nc, in_maps, core_ids=list(range(B)))
    return np.array(
        [np.asarray(out.results[b]["out"]).reshape(()) for b in range(B)],
        dtype=np.float32,
    )


# revision 17
# speedup vs baseline: 3.3135x; 1.0040x over previous
"""Distributed Trainium2 Bass kernel for the associative-embedding (AE) loss.

Problem: per image b (B=8), two tag maps (tm0 [J,256,256], tm1 [J,512,512]),
keypoints kps [NH, 3*J] (x, y, vis interleaved, NH=30 humans, J=17 joints).
Per level: gather tag values at (j, x, y), masked per-human mean, pull loss
(masked squared deviation / num_humans) + push loss (pairwise Gaussian of
means / num_humans^2).  Output: per-image loss [B] (sum over both levels).

Strategy: pure data-parallel over B across 8 NeuronCores (core b handles
image b).  The loss touches only NH*J = 510 elements of each tag map, so
instead of streaming the 178 MB of tag maps, each core computes flat gather
indices on-chip from the keypoint data and pulls exactly 1020 scalars out
of DRAM via 8 indirect (SWDGE) DMAs of 128 single-element descriptors each
(HW indirect DMA = one descriptor per out partition row).  The gathered
values live in a [128, 8] chunk layout; one-hot matrices passed from the
host let the tensor engine reduce that layout directly into per-human
sufficient statistics (sum of masked vals, sum of masked vals^2), since
pull = sum(m v^2) - sv*avg.  Per-chunk stat products and matmuls are
pipelined under the remaining gathers.  The push loss uses a 32x32 DVE
stream transpose for the pairwise mean differences.  Per-core output is a
single scalar; the host stacks the 8 scalars into the final [8] vector.
"""

import numpy as np

B = 8
NH = 30
J = 17
H0 = W0 = 256
H1 = W1 = 512
N0 = J * H0 * W0
N1 = J * H1 * W1
NTOT = N0 + N1
NR = 2 * J * NH           # 1020 gathered elements
NC = 8                    # chunks of 128 (r = c*128 + p)
NI = 128 * NC
BIG = 1.0e9               # pad avg rows 30/31 -> exp(-BIG^2/2) = 0

_CACHE = {}

# ---------------------------------------------------------------------------
# host-side constants: chunk layout r = c*128 + p, f = r // NH, nh = r % NH
# ---------------------------------------------------------------------------


def _host_constants():
    if "consts" in _CACHE:
        return _CACHE["consts"]
    r = np.arange(NI)
    valid = r < NR
    f = np.where(valid, r // NH, 0)
    nh = np.where(valid, r % NH, 0)
    lvl = f // J
    j = f % J
    wmul = np.where(valid, np.where(lvl == 0, W0, W1), 0)
    base = np.where(valid, np.where(lvl == 0, j * H0 * W0, N0 + j * H1 * W1), 0)

    def chunkify(a):  # [NI] -> [128, NC]
        return np.ascontiguousarray(a.reshape(NC, 128).T)

    kcw = chunkify(wmul).astype(np.int32)
    kcb = chunkify(base).astype(np.int32)
    L0 = chunkify((valid & (lvl == 0)).astype(np.float32))
    L1 = chunkify((valid & (lvl == 1)).astype(np.float32))
    E = np.zeros((128, NC * NH), dtype=np.float32)
    for c in range(NC):
        rr = np.arange(c * 128, (c + 1) * 128)
        ok = rr < NR
        E[ok, c * NH + (rr[ok] % NH)] = 1.0
    cf = np.concatenate([L0, L1, E], axis=1).astype(np.float32)
    col_x = (lvl * 3 * J + 3 * j).astype(np.int64)
    _CACHE["consts"] = dict(
        kcw=kcw, kcb=kcb, cf=cf, nh=nh, col_x=col_x, valid=valid,
        chunkify=chunkify,
    )
    return _CACHE["consts"]


def make_in_maps(tag_maps0, tag_maps1, kps0, kps1):
    tag_maps0 = np.asarray(tag_maps0, dtype=np.float32)
    tag_maps1 = np.asarray(tag_maps1, dtype=np.float32)
    kps0 = np.asarray(kps0, dtype=np.int32)
    kps1 = np.asarray(kps1, dtype=np.int32)
    C = _host_constants()
    nh, col_x, valid = C["nh"], C["col_x"], C["valid"]
    chunkify = C["chunkify"]
    in_maps = []
    for b in range(B):
        tm = np.concatenate(
            [tag_maps0[b].ravel(), tag_maps1[b].ravel()]
        ).reshape(NTOT, 1)
        kp = np.concatenate([kps0[b], kps1[b]], axis=1)  # [30, 102]
        xs = np.zeros(NI, np.int32)
        ys = np.zeros(NI, np.int32)
        vs = np.zeros(NI, np.int32)
        xs[valid] = kp[nh[valid], col_x[valid]]
        ys[valid] = kp[nh[valid], col_x[valid] + 1]
        vs[valid] = kp[nh[valid], col_x[valid] + 2]
        kpg = np.stack(
            [chunkify(xs), chunkify(ys), chunkify(vs)], axis=2
        ).reshape(128, 3 * NC)
        ki = np.concatenate([kpg, C["kcw"], C["kcb"]], axis=1)  # [128, 40]
        in_maps.append({"tm": tm, "kp": kp, "ki": ki, "cf": C["cf"]})
    return in_maps


# ---------------------------------------------------------------------------
# device kernel
# ---------------------------------------------------------------------------


def _build_nc():
    from concourse import bacc, mybir
    import concourse.tile as tile
    from concourse.bass import IndirectOffsetOnAxis

    f32 = mybir.dt.float32
    i32 = mybir.dt.int32
    Alu = mybir.AluOpType
    X = mybir.AxisListType.X

    nc = bacc.Bacc()
    TM = nc.declare_dram_parameter("tm", [NTOT, 1], f32, isOutput=False)
    KP = nc.declare_dram_parameter("kp", [NH, 6 * J], i32, isOutput=False)
    KI = nc.declare_dram_parameter("ki", [128, 5 * NC], i32, isOutput=False)
    CF = nc.declare_dram_parameter(
        "cf", [128, 2 * NC + NC * NH], f32, isOutput=False
    )
    OUT = nc.declare_dram_parameter("out", [1, 1], f32, isOutput=True)

    with tile.TileContext(nc) as tc:
        with (
            tc.tile_pool(name="sb", bufs=1) as sb,
            tc.tile_pool(name="pp", bufs=1, space="PSUM") as pp,
        ):
            kt = sb.tile([NH, 6 * J], i32)
            ki = sb.tile([128, 5 * NC], i32)
            cf = sb.tile([128, 2 * NC + NC * NH], f32)
            idxc = sb.tile([128, NC], i32)
            S = sb.tile([128, NC], f32)
            maskg = sb.tile([128, NC], f32)
            T = sb.tile([128, 4 * NC], f32)
            sgq = sb.tile([128, 2 * NC], f32)
            maskf = sb.tile([NH, 2 * J], f32)
            cnt = sb.tile([NH, 2], f32)
            den = sb.tile([NH, 2], f32)
            rden = sb.tile([NH, 2], f32)
            st = sb.tile([NH, 4], f32)
            avg0 = sb.tile([NH, 2], f32)
            u = sb.tile([NH, 2], f32)
            avg32 = sb.tile([32, 2], f32)
            avgsrc = sb.tile([32, 64], f32)
            avgT = sb.tile([32, 64], f32)
            d2 = sb.tile([NH, 64], f32)
            pm = sb.tile([NH, 64], f32)
            pack = sb.tile([NH, 6], f32)
            ones = sb.tile([NH, 1], f32)
            warm = sb.tile([1, 1], f32)
            sums = sb.tile([1, 6], f32)
            rec = sb.tile([1, 6], f32)
            m1 = sb.tile([1, 4], f32)
            res = sb.tile([1, 1], f32)
            ps_st = pp.tile([NH, 4], f32)
            ps_f = pp.tile([1, 6], f32)

            # Warm the ACT Exp table during startup.
            nc.vector.memset(warm[:], 0.0)
            nc.scalar.activation(
                warm[:], warm[:], mybir.ActivationFunctionType.Exp
            )

            # Inputs in; ki first (it gates the gathers).
            nc.sync.dma_start(ki[:], KI[:])
            nc.gpsimd.dma_start(kt[:], KP[:])
            nc.gpsimd.dma_start(cf[:], CF[:])

            # Gather indices in chunk layout: idx = x*W + y + base.
            xg = ki[:, 0 : 3 * NC : 3]
            yg = ki[:, 1 : 3 * NC : 3]
            vg = ki[:, 2 : 3 * NC : 3]
            nc.vector.tensor_tensor(
                out=idxc[:], in0=xg, in1=ki[:, 3 * NC : 4 * NC], op=Alu.mult
            )
            nc.vector.tensor_tensor(out=idxc[:], in0=idxc[:], in1=yg, op=Alu.add)
            nc.vector.tensor_tensor(
                out=idxc[:], in0=idxc[:], in1=ki[:, 4 * NC : 5 * NC], op=Alu.add
            )
            nc.vector.tensor_scalar(
                out=maskg[:], in0=vg, scalar1=0, scalar2=None, op0=Alu.is_gt
            )

            # The only touch of the big tag maps: 1020 scalars in 8 indirect
            # DMAs; per-chunk stat products + matmuls pipeline right behind
            # each chunk's gather.
            L0 = cf[:, 0:NC]
            L1 = cf[:, NC : 2 * NC]
            for c in range(NC):
                nc.gpsimd.indirect_dma_start(
                    out=S[:, c : c + 1],
                    out_offset=None,
                    in_=TM[:],
                    in_offset=IndirectOffsetOnAxis(ap=idxc[:, c : c + 1], axis=0),
                )
                cs = slice(2 * c, 2 * c + 1)
                cs2 = slice(2 * c + 1, 2 * c + 2)
                nc.vector.tensor_tensor(
                    out=sgq[:, cs], in0=S[:, c : c + 1], in1=maskg[:, c : c + 1],
                    op=Alu.mult,
                )
                nc.vector.tensor_tensor(
                    out=sgq[:, cs2], in0=sgq[:, cs], in1=S[:, c : c + 1],
                    op=Alu.mult,
                )
                # T[:, {c, 8+c}] = sg * [L0_c, L1_c];
                # T[:, {16+c, 24+c}] = sg2 * [L0_c, L1_c]
                lc = cf[:, c : 2 * NC : NC]
                nc.vector.tensor_tensor(
                    out=T[:, c : NC + c + 1 : NC],
                    in0=sgq[:, cs].to_broadcast([128, 2]),
                    in1=lc,
                    op=Alu.mult,
                )
                nc.vector.tensor_tensor(
                    out=T[:, 2 * NC + c : 3 * NC + c + 1 : NC],
                    in0=sgq[:, cs2].to_broadcast([128, 2]),
                    in1=lc,
                    op=Alu.mult,
                )
                nc.tensor.matmul(
                    ps_st[:],
                    lhsT=cf[:, 2 * NC + c * NH : 2 * NC + (c + 1) * NH],
                    rhs=T[:, c : 4 * NC : NC],
                    start=(c == 0),
                    stop=(c == NC - 1),
                )

            # Joint-count path (independent of gathers; fills DVE idle time).
            vis = kt[:, 2 : 6 * J : 3]
            nc.vector.tensor_scalar(
                out=maskf[:], in0=vis, scalar1=0, scalar2=None, op0=Alu.is_gt
            )
            nc.vector.reduce_sum(
                out=cnt[:], in_=maskf[:].rearrange("p (l j) -> p l j", l=2), axis=X
            )
            nc.vector.tensor_scalar(
                out=den[:], in0=cnt[:], scalar1=1.0, scalar2=None, op0=Alu.max
            )
            nc.vector.reciprocal(rden[:], den[:])
            nc.vector.tensor_scalar(
                out=pack[:, 4:6], in0=cnt[:], scalar1=0.0, scalar2=None, op0=Alu.is_gt
            )
            nc.vector.memset(avg32[:], BIG)
            nc.vector.memset(ones[:], 1.0)

            # Per-human stats -> averages, pull (read stats from PSUM).
            sv = ps_st[:, 0:2]
            s2 = ps_st[:, 2:4]
            nc.vector.tensor_tensor(out=avg0[:], in0=sv, in1=rden[:], op=Alu.mult)
            nc.vector.tensor_tensor(
                out=avg32[0:NH, :], in0=avg0[:], in1=pack[:, 4:6], op=Alu.mult
            )
            # pull = s2 - sv*avg0 (zero when cnt == 0 since sv = s2 = 0)
            nc.vector.tensor_tensor(out=u[:], in0=sv, in1=avg0[:], op=Alu.mult)
            nc.vector.tensor_tensor(
                out=pack[:, 0:2], in0=s2, in1=u[:], op=Alu.subtract
            )

            # Push: pairwise means via 32x32 block stream transpose.
            nc.vector.tensor_copy(
                out=avgsrc[:].rearrange("p (l j) -> p l j", l=2),
                in_=avg32[:].to_broadcast([32, 2, 32]),
            )
            nc.vector.transpose(avgT[:], avgsrc[:])
            nc.vector.tensor_tensor(
                out=d2[:].rearrange("p (l j) -> p l j", l=2),
                in0=avgT[0:NH, :].rearrange("p (l j) -> p l j", l=2),
                in1=avg32[0:NH, :].to_broadcast([NH, 2, 32]),
                op=Alu.subtract,
            )
            nc.vector.tensor_tensor(out=d2[:], in0=d2[:], in1=d2[:], op=Alu.mult)
            nc.scalar.activation(
                pm[:], d2[:], mybir.ActivationFunctionType.Exp, scale=-0.5
            )
            nc.vector.reduce_sum(
                out=pack[:, 2:4],
                in_=pm[:].rearrange("p (l j) -> p l j", l=2),
                axis=X,
            )

            # Column sums over the 30 humans via PE, then the final scalar.
            nc.tensor.matmul(
                ps_f[:], lhsT=ones[:], rhs=pack[:], start=True, stop=True
            )
            nc.vector.reciprocal(rec[:], ps_f[:])
            nc.vector.tensor_tensor(
                out=m1[:, 0:2], in0=ps_f[:, 0:2], in1=rec[:, 4:6], op=Alu.mult
            )
            nc.vector.tensor_tensor(
                out=m1[:, 2:4], in0=ps_f[:, 2:4], in1=rec[:, 4:6], op=Alu.mult
            )
            nc.vector.tensor_tensor(
                out=m1[:, 2:4], in0=m1[:, 2:4], in1=rec[:, 4:6], op=Alu.mult
            )
            nc.vector.reduce_sum(out=res[:], in_=m1[:], axis=X)

            nc.sync.dma_start(OUT[:], res[:])

    nc.finalize()
    return nc


def _get_nc():
    if "nc" not in _CACHE:
        _CACHE["nc"] = _build_nc()
    return _CACHE["nc"]


def kernel(tag_maps0, tag_maps1, kps0, kps1):
    from concourse.bass_utils import run_bass_kernel_spmd

    nc = _get_nc()
    in_maps = make_in_maps(tag_maps0, tag_maps1, kps0, kps1)
    out = run_bass_kernel_spmd(nc, in_maps, core_ids=list(range(B)))
    return np.array(
        [np.asarray(out.results[b]["out"]).reshape(()) for b in range(B)],
        dtype=np.float32,
    )
